# revision 68
# baseline (speedup 1.0000x reference)
"""Causal self-attention with RoPE on 8 trn2 NeuronCores (v2, bf16).

Sharding: core c handles batch b = c//2 and head-half = c%2 (8 of 16 heads).
Each core computes its heads' attention output and a partial output
projection (row-slice of Wp); host sums the two partials per batch.

v2 dataflow (all matmuls bf16, single projection pass, no DRAM scratch):
  xT   [C=1024, T=2048] bf16  x[b] transposed, streamed once (batched DMA)
  qT/kT per pair p: [128 = 2 heads x 64 dims (deinterleaved), T] bf16
  v    [128, 16, 8, 65] bf16  keys on partitions, 65th col = ones
  S^T  [tk, tq] PSUM tiles -> exp on ACT -> pt bf16 SBUF (causal-trimmed)
  PV   pvps[s] [65, 512] accumulated over tk; row 64 = softmax denominator
  norm recip (DVE) -> ones-column broadcast matmul (PE) -> yn mult
  y^T  [128, T] bf16 per pair kept in SBUF -> phase B -> out [T,1024] f32
"""

import math
import sys

import numpy as np

for _p in ("/opt/trn_rl_repo",):
    if _p not in sys.path:
        sys.path.insert(0, _p)

B, T, C, H = 4, 2048, 1024, 16
D = C // H           # 64
HALF = D // 2        # 32
NCORES = 8
HPC = H // 2         # heads per core
NPAIR = HPC // 2     # head pairs per core
CK = C // 128        # 8 C-chunks
TCH = 512            # tq / T chunk width
NTCH = T // TCH     # 4
NTK = T // 128       # 16 tk tiles


def build_nc():
    """Build the single-core SPMD Bass program (same NEFF on all 8 cores)."""
    import concourse.bass as bass
    import concourse.mybir as mybir
    import concourse.tile as tile
    from concourse.bass import ds, ts

    fp32 = mybir.dt.float32
    f32r = mybir.dt.float32r
    bf16 = mybir.dt.bfloat16
    Alu = mybir.AluOpType
    Act = mybir.ActivationFunctionType

    nc = bass.Bass("TRN2", target_bir_lowering=False, debug=False,
                   num_devices=NCORES)

    def din(name, shape, dt=None):
        return nc.dram_tensor(
            name, list(shape), dt or fp32, kind="ExternalInput").ap()

    xT_d = din("xT", (C, T), bf16)
    wq_d = din("wq", (C, HPC * D), bf16)
    wk_d = din("wk", (C, HPC * D), bf16)
    wv_d = din("wv", (C, HPC * D), bf16)
    wp_d = din("wp", (HPC * D, C), bf16)
    cpak_d = din("cpak", (128, 8 + HPC * D + C))
    hpak_d = din("hpak", (128, 456 + 2 * T), bf16)
    out_d = nc.dram_tensor("out", [T, C], fp32, kind="ExternalOutput").ap()

    with tile.TileContext(nc) as tc:
        with (
            nc.allow_low_precision(reason="bf16 kernel, rel-err budget 2e-2"),
            tc.tile_pool(name="small", bufs=1) as small,
            tc.tile_pool(name="big", bufs=1) as big,
            tc.tile_pool(name="qk", bufs=2 * NPAIR) as qkp,
            tc.tile_pool(name="yt", bufs=NPAIR) as ytp,
            tc.tile_pool(name="xt", bufs=3) as xtp,
            tc.tile_pool(name="raw", bufs=8) as rawp,
            tc.tile_pool(name="pt", bufs=4) as ptp,
            tc.tile_pool(name="rs", bufs=4) as rsp,
            tc.tile_pool(name="osb", bufs=4) as osbp,
            tc.tile_pool(name="psS", bufs=2, space="PSUM") as psS,
            tc.tile_pool(name="psV", bufs=2, space="PSUM") as psV,
            tc.tile_pool(name="psC", bufs=2, space="PSUM") as psC,
        ):
            # ---- long-lived constants (packed: 2 DMAs, issued late) ----
            # fp32 pack: bq_r[4] | bk_r[4] | bv_b[512] | bp_b[1024]
            # bf16 pack: pswap[128] | ones8[8] | ones64row[64] | tri[128]
            #            | ropeA[2048] | ropeB[2048]
            wv_sb = big.tile([128, CK, HPC * D], bf16)
            wq_sb = big.tile([128, CK, HPC * D], bf16)
            wk_sb = big.tile([128, CK, HPC * D], bf16)
            wp_sb = big.tile([128, NPAIR, C], bf16)
            cpak_sb = small.tile([128, 8 + HPC * D + C], fp32)
            bq_sb = cpak_sb[:, 0:NPAIR]
            bk_sb = cpak_sb[:, NPAIR:2 * NPAIR]
            bvb_sb = cpak_sb[:, 8:8 + HPC * D]
            bpb_sb = cpak_sb[:, 8 + HPC * D:8 + HPC * D + C]
            hpak_sb = small.tile([128, 456 + 2 * T], bf16)
            pswap_sb = hpak_sb[:, 0:128]
            ones8_sb = hpak_sb[:, 128:128 + HPC]
            ones64_sb = hpak_sb[0:1, 136:136 + D]
            negL_sb = hpak_sb[:, 200:328]
            ident_sb = hpak_sb[:, 328:456]
            ropeA_sb = hpak_sb[:, 456:456 + T]
            ropeB_sb = hpak_sb[:, 456 + T:456 + 2 * T]

            v_sb = big.tile([128, NTK, HPC, 65], bf16)

            qT = {}
            kT = {}
            for p in range(NPAIR):
                qT[p] = qkp.tile([128, T], bf16, tag="qk", name=f"qT{p}")
                kT[p] = qkp.tile([128, T], bf16, tag="qk", name=f"kT{p}")
            yT = {}
            for p in range(NPAIR):
                yT[p] = ytp.tile([128, T], bf16, tag="yt", name=f"yT{p}")

            xTr = xT_d.rearrange("(k q) t -> q k t", q=128)

            def proj_units(tcid, qk_first=False):
                """Projection for T-chunk tcid as a list of filler units."""
                xt = xtp.tile([128, CK, TCH], bf16, tag="xt",
                              name=f"xt{tcid}")

                def u_dma():
                    if tcid == 0:
                        nc.sync.dma_start(
                            xt[:, 0:CK // 2, :],
                            xTr[:, 0:CK // 2, ts(tcid, TCH)])
                        nc.sync.dma_start(
                            xt[:, CK // 2:, :],
                            xTr[:, CK // 2:, ts(tcid, TCH)])
                    else:
                        nc.sync.dma_start(xt[:], xTr[:, :, ts(tcid, TCH)])

                def u_v(tt):
                    tk = tcid * 4 + tt
                    pv = psC.tile([128, TCH], fp32, tag="c")
                    for kc in range(CK):
                        nc.tensor.matmul(
                            pv[:], xt[:, kc, ts(tt, 128)], wv_sb[:, kc, :],
                            start=(kc == 0), stop=(kc == CK - 1))
                    nc.vector.tensor_tensor(
                        out=v_sb[:, tk, :, 0:64],
                        in0=pv[:].rearrange("p (h e) -> p h e", e=64),
                        in1=bvb_sb.rearrange("p (h e) -> p h e", e=64),
                        op=Alu.add)

                def u_qk(p, qk):
                    w_sb, b_sb, dst = (
                        (wq_sb, bq_sb, qT[p]) if qk == "q"
                        else (wk_sb, bk_sb, kT[p]))
                    pq = psC.tile([128, TCH], fp32, tag="c")
                    for kc in range(CK):
                        nc.tensor.matmul(
                            pq[:], w_sb[:, kc, ts(p, 128)], xt[:, kc, :],
                            start=(kc == 0), stop=(kc == CK - 1))
                    raw = rawp.tile([128, TCH], bf16, tag="raw")
                    nc.vector.tensor_scalar_add(
                        raw[:], pq[:], b_sb[:, p:p + 1])
                    psw = psC.tile([128, TCH], fp32, tag="c")
                    nc.tensor.matmul(
                        psw[:], pswap_sb, raw[:], start=True, stop=True)
                    tt_ = rawp.tile([128, TCH], bf16, tag="t")
                    nc.gpsimd.tensor_tensor(
                        out=tt_[:], in0=raw[:],
                        in1=ropeA_sb[:, ts(tcid, TCH)], op=Alu.mult)
                    uu = rawp.tile([128, TCH], bf16, tag="u")
                    nc.vector.tensor_tensor(
                        out=uu[:], in0=psw[:],
                        in1=ropeB_sb[:, ts(tcid, TCH)], op=Alu.mult)
                    nc.gpsimd.tensor_tensor(
                        out=dst[:, ts(tcid, TCH)], in0=tt_[:],
                        in1=uu[:], op=Alu.add)

                units = [u_dma]
                qks = [lambda p=p, qk=qk: u_qk(p, qk)
                       for p in range(NPAIR) for qk in ("q", "k")]
                vs = [lambda tt=tt: u_v(tt) for tt in range(4)]
                if qk_first:
                    units += qks[:2] + vs + qks[2:]
                else:
                    units += vs + qks
                return units

            def attention_col(p, j, tick=lambda: None):
                """Attention for pair p, query chunk j (keys 0..512(j+1))."""
                n_tk = 4 * j + 4
                pvps = [psV.tile([65, TCH], fp32, tag="pv",
                                 name=f"pv{p}_{j}_{s_}")
                        for s_ in range(2)]
                for g in range(n_tk // 2):
                    for s in range(2):
                        hs = 2 * p + s
                        row = ds(64 * s, 64)
                        sp = psS.tile([128, 2, TCH], fp32, tag="sg")
                        pt = ptp.tile([128, 2, TCH], bf16, tag="pt")
                        diag = 2 * g >= 4 * j
                        for ti in (0, 1):
                            tk = 2 * g + ti
                            i = tk - 4 * j
                            lo = 128 * i if i > 0 else 0
                            nc.tensor.matmul(
                                sp[:, ti, lo:TCH],
                                kT[p][row, ts(tk, 128)],
                                qT[p][row, ds(TCH * j + lo, TCH - lo)],
                                start=True, stop=not diag)
                            if diag:
                                # additive causal mask: accumulate -1e9
                                # into the diagonal 128-block (ident.T@negL)
                                nc.tensor.matmul(
                                    sp[:, ti, ds(128 * i, 128)],
                                    ident_sb, negL_sb,
                                    start=False, stop=True)
                        if not diag:
                            nc.scalar.activation(
                                pt[:], sp[:], Act.Exp,
                                scale=1.0 / math.sqrt(D))
                        else:
                            for ti in (0, 1):
                                tk = 2 * g + ti
                                i = tk - 4 * j
                                lo = 128 * i if i > 0 else 0
                                nc.scalar.activation(
                                    pt[:, ti, lo:TCH], sp[:, ti, lo:TCH],
                                    Act.Exp, scale=1.0 / math.sqrt(D))
                        for ti in (0, 1):
                            tk = 2 * g + ti
                            i = tk - 4 * j
                            lo = 128 * i if i > 0 else 0
                            nc.tensor.matmul(
                                pvps[s][:, lo:TCH],
                                v_sb[:, tk, hs, :],
                                pt[:, ti, lo:TCH],
                                start=(tk == 0),
                                stop=(tk == n_tk - 1))
                        tick()
                for s in range(2):
                    # recip straight off PSUM, in parallel with the copy
                    # that releases the PSUM bank (pairs pipeline via psV)
                    r_sb = rsp.tile([1, TCH], bf16, tag="r")
                    nc.vector.reciprocal(
                        out=r_sb[:], in_=pvps[s][64:65, :])
                    ysb = rawp.tile([64, TCH], bf16, tag="ysb")
                    nc.vector.tensor_copy(out=ysb[:], in_=pvps[s][0:64, :])
                    rb = psC.tile([64, TCH], fp32, tag="c")
                    nc.tensor.matmul(
                        rb[:], ones64_sb, r_sb[:],
                        start=True, stop=True)
                    nc.vector.tensor_tensor(
                        out=yT[p][ds(64 * s, 64), ts(j, TCH)],
                        in0=ysb[:], in1=rb[:], op=Alu.mult)

            def phase_b_units(t, wide=False):
                """Output projection for T-tile t as 2 micro filler units;
                each half DMAs out as soon as its bias add lands. wide=True
                borrows the (idle, post-attention) psS pool for the
                accumulators so all final tiles pipeline in parallel."""
                osb = osbp.tile([128, C], fp32, tag="osb", name=f"osb{t}")

                def u_half(n):
                    if wide:
                        pot = psS.tile([128, 2, TCH], fp32, tag="sg")
                        po = pot[:, 0, :]
                    else:
                        po = psC.tile([128, TCH], fp32, tag="c")
                    for p in range(NPAIR):
                        nc.tensor.matmul(
                            po[:], yT[p][:, ts(t, 128)],
                            wp_sb[:, p, ts(n, TCH)],
                            start=(p == 0), stop=(p == NPAIR - 1))
                    nc.vector.tensor_tensor(
                        out=osb[:, ts(n, TCH)], in0=po[:],
                        in1=bpb_sb[:, ts(n, TCH)], op=Alu.add)
                    nc.sync.dma_start(
                        out_d[ts(t, 128), ts(n, TCH)], osb[:, ts(n, TCH)])

                return [lambda: u_half(0), lambda: u_half(1)]

            def phase_b(t):
                for u in phase_b_units(t):
                    u()

            # ---- emission ----
            # DMA order: x chunk 0 and wv first (first compute is the v
            # projection), then the other weights/constants as needed.
            pu0 = proj_units(0)
            pu0[0]()                        # xt0 DMA
            # wv in two halves so the first v matmuls start sooner
            wvr = wv_d.rearrange("(k q) f -> q k f", q=128)
            nc.sync.dma_start(wv_sb[:, 0:CK // 2, :], wvr[:, 0:CK // 2, :])
            nc.sync.dma_start(wv_sb[:, CK // 2:, :], wvr[:, CK // 2:, :])
            nc.sync.dma_start(
                wq_sb[:], wq_d.rearrange("(k q) f -> q k f", q=128))
            nc.sync.dma_start(cpak_sb[:], cpak_d[:, :])
            nc.sync.dma_start(hpak_sb[:], hpak_d[:, :])
            nc.sync.dma_start(
                wk_sb[:], wk_d.rearrange("(k q) f -> q k f", q=128))
            nc.vector.tensor_copy(
                out=v_sb[:, :, :, 64:65],
                in_=ones8_sb[:, None, :, None].to_broadcast(
                    (128, NTK, HPC, 1)))
            for u in pu0[1:]:
                u()
            nc.sync.dma_start(
                wp_sb[:], wp_d.rearrange("(p q) n -> q p n", q=128))
            # Filler schedule: projection chunks front-loaded as coarse
            # fillers between attention pairs (they must finish before
            # their own attention column); output-projection rows become
            # micro fillers ticked INSIDE the late, ACT-bound attention
            # columns so PE fill never starves ACT of score tiles.
            pu1, pu2, pu3 = (proj_units(1), proj_units(2), proj_units(3))
            coarse = {
                0: [],
                1: [],
                2: [],
                3: [],
            }
            micro = {
                0: pu1 + pu2,
                1: pu3[:9],
                2: pu3[9:] + [u for t in range(4) for u in phase_b_units(t)],
                3: [u for t in range(4, 12) for u in phase_b_units(t)],
            }
            for j in range(NTCH):
                fill = coarse[j]
                mic = micro[j]
                ticks_total = NPAIR * 2 * (2 * j + 2)
                stride = max(1, ticks_total // max(1, len(mic)))
                cnt = [0]

                def tick():
                    cnt[0] += 1
                    if mic and cnt[0] % stride == 0:
                        mic.pop(0)()

                k = len(fill)
                for p in range(NPAIR):
                    attention_col(p, j, tick)
                    take = (k * (p + 1)) // NPAIR - (k * p) // NPAIR
                    for _ in range(take):
                        fill.pop(0)()
                for u in mic:
                    u()
            for t in range(12, NTK):
                for u in phase_b_units(t, wide=(t % 2 == 1)):
                    u()
    _split_drain_waits(nc, mybir)
    return nc


def _split_drain_waits(nc, mybir, max_w=1):
    """This walrus build allows at most one embedded sync wait per
    instruction (CTRL_NO for drains, S3_LW for matmuls, ...). Hoist all but
    the last wait of every instruction into standalone EventSemaphore
    instructions on the same engine, inserted immediately before it."""
    import bass_rust

    for f in nc.m.functions:
        for blk in f.blocks:
            insts = list(blk.instructions)
            out = []
            changed = False
            for ins in insts:
                si = ins.sync_info
                if si is not None and si.on_wait and len(si.on_wait) > max_w:
                    changed = True
                    waits = list(si.on_wait)
                    extra, keep = waits[:-max_w], waits[-max_w:]
                    for wi, w in enumerate(extra):
                        ev = mybir.InstEventSemaphore(
                            name=f"{ins.name}_w{wi}",
                            engine=ins.engine,
                            ins=[], outs=[],
                            debug=ins.debug,
                            sync_info=bass_rust.SyncInfo(
                                on_wait=[w], on_update=[]),
                        )
                        nc.register_instruction(ev, overwrite=True)
                        out.append(ev)
                    si.on_wait = keep
                    ins.sync_info = si
                out.append(ins)
            if changed:
                blk.instructions = out


def host_inputs(x, Wq, bq, Wk, bk, Wv, bv, Wp, bp):
    """Build the 8 per-core input maps."""
    import ml_dtypes

    bft = ml_dtypes.bfloat16
    half = D // 2
    perm = np.concatenate([np.arange(0, D, 2), np.arange(1, D, 2)])  # even|odd
    pos = np.arange(T, dtype=np.float32)[:, None]
    freqs = np.exp(np.arange(half, dtype=np.float32)
                   * np.float32(-math.log(10000.0) / (half - 1)))[None, :]
    args = pos * freqs                      # [T, 32]
    cos = np.cos(args).astype(np.float32).T   # [32, T]
    sin = np.sin(args).astype(np.float32).T
    ropeA = np.concatenate([cos, cos, cos, cos], 0).astype(bft)   # [128, T]
    ropeB = np.concatenate([-sin, sin, -sin, sin], 0).astype(bft)
    pswap = np.zeros((128, 128), np.float32)
    for blk in range(4):
        b0 = 32 * blk
        src = 32 * (blk ^ 1)
        for i in range(32):
            pswap[b0 + i, src + i] = 1.0
    r_idx = np.arange(128)[:, None]
    c_idx = np.arange(128)[None, :]
    negL = np.where(r_idx > c_idx, -1.0e9, 0.0).astype(bft)
    ident = np.eye(128).astype(bft)

    in_maps = []
    for core in range(NCORES):
        b = core // 2
        h0 = (core % 2) * HPC
        cols = []
        for p in range(NPAIR):
            for hh in range(2):
                h = h0 + 2 * p + hh
                cols.append(h * D + perm)
        cols = np.concatenate(cols)           # deinterleaved q/k columns
        vcols = np.arange(h0 * D, (h0 + HPC) * D)
        bq_r = np.ascontiguousarray(
            bq[cols].reshape(NPAIR, 128).T)   # [128, 4]
        bk_r = np.ascontiguousarray(bk[cols].reshape(NPAIR, 128).T)
        bp_core = bp if core % 2 == 0 else np.zeros_like(bp)
        # fp32 pack: bq_r[4] | bk_r[4] | bv_b[512] | bp_b[1024]
        cpak = np.concatenate([
            bq_r, bk_r,
            np.broadcast_to(bv[vcols], (128, HPC * D)),
            np.broadcast_to(bp_core, (128, C)),
        ], axis=1).astype(np.float32)
        # bf16 pack: pswap[128] | ones8[8] | ones64row[64] | tri[128]
        #            | ropeA[2048] | ropeB[2048]  (ones64 row 0 only used)
        hpak = np.concatenate([
            pswap.astype(bft),
            np.ones((128, HPC + D), bft),
            negL, ident, ropeA, ropeB,
        ], axis=1).astype(bft)
        in_maps.append({
            "xT": np.ascontiguousarray(x[b].T).astype(bft),
            "wq": np.ascontiguousarray(Wq[:, cols]).astype(bft),
            "wk": np.ascontiguousarray(Wk[:, cols]).astype(bft),
            "wv": np.ascontiguousarray(Wv[:, vcols]).astype(bft),
            "wp": np.ascontiguousarray(Wp[vcols, :]).astype(bft),
            "cpak": cpak,
            "hpak": hpak,
        })
    return in_maps


_CACHE = {}
_PROFILE = False


def kernel(**inputs) -> np.ndarray:
    x = np.asarray(inputs["x"], np.float32)
    in_maps = host_inputs(
        x, *(np.asarray(inputs[k], np.float32) for k in
             ("Wq", "bq", "Wk", "bk", "Wv", "bv", "Wp", "bp")))
    from concourse.bass_utils import run_bass_kernel_spmd
    if "nc" not in _CACHE:
        _CACHE["nc"] = build_nc()
    bkr = run_bass_kernel_spmd(
        _CACHE["nc"], in_maps, core_ids=list(range(NCORES)),
        trace=_PROFILE)
    _CACHE["last"] = bkr
    res = bkr.results
    out = np.empty((B, T, C), np.float32)
    for b in range(B):
        out[b] = res[2 * b]["out"] + res[2 * b + 1]["out"]
    return out


# revision 75
# speedup vs baseline: 1.4146x; 1.4146x over previous
"""Causal self-attention with RoPE on 8 trn2 NeuronCores (v2, bf16).

Sharding: core c handles batch b = c//2 and head-half = c%2 (8 of 16 heads).
Each core computes its heads' attention output and a partial output
projection (row-slice of Wp); host sums the two partials per batch.

v2 dataflow (all matmuls bf16, single projection pass, no DRAM scratch):
  xT   [C=1024, T=2048] bf16  x[b] transposed, streamed once (batched DMA)
  qT/kT per pair p: [128 = 2 heads x 64 dims (deinterleaved), T] bf16
  v    [128, 16, 8, 65] bf16  keys on partitions, 65th col = ones
  S^T  [tk, tq] PSUM tiles -> exp on ACT -> pt bf16 SBUF (causal-trimmed)
  PV   pvps[s] [65, 512] accumulated over tk; row 64 = softmax denominator
  norm recip (DVE) -> ones-column broadcast matmul (PE) -> yn mult
  y^T  [128, T] bf16 per pair kept in SBUF -> phase B -> out [T,1024] f32
"""

import math
import sys

import numpy as np

for _p in ("/opt/trn_rl_repo",):
    if _p not in sys.path:
        sys.path.insert(0, _p)

B, T, C, H = 4, 2048, 1024, 16
D = C // H           # 64
HALF = D // 2        # 32
NCORES = 8
HPC = H // 2         # heads per core
NPAIR = HPC // 2     # head pairs per core
CK = C // 128        # 8 C-chunks
TCH = 512            # tq / T chunk width
NTCH = T // TCH     # 4
NTK = T // 128       # 16 tk tiles


def build_nc():
    """Build the single-core SPMD Bass program (same NEFF on all 8 cores)."""
    import concourse.bass as bass
    import concourse.mybir as mybir
    import concourse.tile as tile
    from concourse.bass import ds, ts

    fp32 = mybir.dt.float32
    f32r = mybir.dt.float32r
    bf16 = mybir.dt.bfloat16
    Alu = mybir.AluOpType
    Act = mybir.ActivationFunctionType

    nc = bass.Bass("TRN2", target_bir_lowering=False, debug=False,
                   num_devices=NCORES)

    def din(name, shape, dt=None):
        return nc.dram_tensor(
            name, list(shape), dt or fp32, kind="ExternalInput").ap()

    xT_d = din("xT", (C, T), bf16)
    wq_d = din("wq", (C, HPC * D), bf16)
    wk_d = din("wk", (C, HPC * D), bf16)
    wv_d = din("wv", (C, HPC * D), bf16)
    wp_d = din("wp", (HPC * D, C), bf16)
    cpak_d = din("cpak", (128, 8 + HPC * D + C))
    hpak_d = din("hpak", (128, 456 + 2 * T), bf16)
    out_d = nc.dram_tensor("out", [T, C], fp32, kind="ExternalOutput").ap()

    with tile.TileContext(nc) as tc:
        with (
            nc.allow_low_precision(reason="bf16 kernel, rel-err budget 2e-2"),
            tc.tile_pool(name="small", bufs=1) as small,
            tc.tile_pool(name="big", bufs=1) as big,
            tc.tile_pool(name="qk", bufs=2 * NPAIR) as qkp,
            tc.tile_pool(name="yt", bufs=NPAIR) as ytp,
            tc.tile_pool(name="xt", bufs=3) as xtp,
            tc.tile_pool(name="raw", bufs=8) as rawp,
            tc.tile_pool(name="pt", bufs=4) as ptp,
            tc.tile_pool(name="rs", bufs=4) as rsp,
            tc.tile_pool(name="osb", bufs=4) as osbp,
            tc.tile_pool(name="psS", bufs=2, space="PSUM") as psS,
            tc.tile_pool(name="psV", bufs=2, space="PSUM") as psV,
            tc.tile_pool(name="psC", bufs=2, space="PSUM") as psC,
        ):
            # ---- long-lived constants (packed: 2 DMAs, issued late) ----
            # fp32 pack: bq_r[4] | bk_r[4] | bv_b[512] | bp_b[1024]
            # bf16 pack: pswap[128] | ones8[8] | ones64row[64] | tri[128]
            #            | ropeA[2048] | ropeB[2048]
            wv_sb = big.tile([128, CK, HPC * D], bf16)
            wq_sb = big.tile([128, CK, HPC * D], bf16)
            wk_sb = big.tile([128, CK, HPC * D], bf16)
            wp_sb = big.tile([128, NPAIR, C], bf16)
            cpak_sb = small.tile([128, 8 + HPC * D + C], fp32)
            bq_sb = cpak_sb[:, 0:NPAIR]
            bk_sb = cpak_sb[:, NPAIR:2 * NPAIR]
            bvb_sb = cpak_sb[:, 8:8 + HPC * D]
            bpb_sb = cpak_sb[:, 8 + HPC * D:8 + HPC * D + C]
            hpak_sb = small.tile([128, 456 + 2 * T], bf16)
            pswap_sb = hpak_sb[:, 0:128]
            ones8_sb = hpak_sb[:, 128:128 + HPC]
            ones64_sb = hpak_sb[0:1, 136:136 + D]
            negL_sb = hpak_sb[:, 200:328]
            ident_sb = hpak_sb[:, 328:456]
            ropeA_sb = hpak_sb[:, 456:456 + T]
            ropeB_sb = hpak_sb[:, 456 + T:456 + 2 * T]

            v_sb = big.tile([128, NTK, HPC, 65], bf16)

            qT = {}
            kT = {}
            for p in range(NPAIR):
                qT[p] = qkp.tile([128, T], bf16, tag="qk", name=f"qT{p}")
                kT[p] = qkp.tile([128, T], bf16, tag="qk", name=f"kT{p}")
            yT = {}
            for p in range(NPAIR):
                yT[p] = ytp.tile([128, T], bf16, tag="yt", name=f"yT{p}")

            xTr = xT_d.rearrange("(k q) t -> q k t", q=128)

            def proj_units(tcid, qk_first=False):
                """Projection for T-chunk tcid as a list of filler units."""
                xt = xtp.tile([128, CK, TCH], bf16, tag="xt",
                              name=f"xt{tcid}")

                def u_dma():
                    if tcid == 0:
                        nc.sync.dma_start(
                            xt[:, 0:CK // 2, :],
                            xTr[:, 0:CK // 2, ts(tcid, TCH)])
                        nc.sync.dma_start(
                            xt[:, CK // 2:, :],
                            xTr[:, CK // 2:, ts(tcid, TCH)])
                    else:
                        nc.sync.dma_start(xt[:], xTr[:, :, ts(tcid, TCH)])

                def u_v(tt):
                    tk = tcid * 4 + tt
                    pv = psC.tile([128, TCH], fp32, tag="c")
                    for kc in range(CK):
                        nc.tensor.matmul(
                            pv[:], xt[:, kc, ts(tt, 128)], wv_sb[:, kc, :],
                            start=(kc == 0), stop=(kc == CK - 1))
                    nc.vector.tensor_tensor(
                        out=v_sb[:, tk, :, 0:64],
                        in0=pv[:].rearrange("p (h e) -> p h e", e=64),
                        in1=bvb_sb.rearrange("p (h e) -> p h e", e=64),
                        op=Alu.add)

                def u_qk(p, qk):
                    w_sb, b_sb, dst = (
                        (wq_sb, bq_sb, qT[p]) if qk == "q"
                        else (wk_sb, bk_sb, kT[p]))
                    pq = psC.tile([128, TCH], fp32, tag="c")
                    for kc in range(CK):
                        nc.tensor.matmul(
                            pq[:], w_sb[:, kc, ts(p, 128)], xt[:, kc, :],
                            start=(kc == 0), stop=(kc == CK - 1))
                    raw = rawp.tile([128, TCH], bf16, tag="raw")
                    nc.vector.tensor_scalar_add(
                        raw[:], pq[:], b_sb[:, p:p + 1])
                    psw = psC.tile([128, TCH], fp32, tag="c")
                    nc.tensor.matmul(
                        psw[:], pswap_sb, raw[:], start=True, stop=True)
                    tt_ = rawp.tile([128, TCH], bf16, tag="t")
                    nc.gpsimd.tensor_tensor(
                        out=tt_[:], in0=raw[:],
                        in1=ropeA_sb[:, ts(tcid, TCH)], op=Alu.mult)
                    uu = rawp.tile([128, TCH], bf16, tag="u")
                    nc.vector.tensor_tensor(
                        out=uu[:], in0=psw[:],
                        in1=ropeB_sb[:, ts(tcid, TCH)], op=Alu.mult)
                    nc.gpsimd.tensor_tensor(
                        out=dst[:, ts(tcid, TCH)], in0=tt_[:],
                        in1=uu[:], op=Alu.add)

                units = [u_dma]
                qks = [lambda p=p, qk=qk: u_qk(p, qk)
                       for p in range(NPAIR) for qk in ("q", "k")]
                vs = [lambda tt=tt: u_v(tt) for tt in range(4)]
                if qk_first:
                    units += qks[:2] + vs + qks[2:]
                else:
                    units += vs + qks
                return units

            def attention_col(p, j, tick=lambda: None):
                """Attention for pair p, query chunk j (keys 0..512(j+1))."""
                n_tk = 4 * j + 4
                pvps = [psV.tile([65, TCH], fp32, tag="pv",
                                 name=f"pv{p}_{j}_{s_}")
                        for s_ in range(2)]
                for tk in range(n_tk):
                    i = tk - 4 * j
                    lo = 128 * i if i > 0 else 0
                    diag = i >= 0
                    sp = psS.tile([128, 2, TCH], fp32, tag="sg")
                    pt = ptp.tile([128, 2, TCH], bf16, tag="pt")
                    for s in range(2):
                        row = ds(64 * s, 64)
                        nc.tensor.matmul(
                            sp[:, s, lo:TCH],
                            kT[p][row, ts(tk, 128)],
                            qT[p][row, ds(TCH * j + lo, TCH - lo)],
                            start=True, stop=not diag)
                        if diag:
                            # additive causal mask: accumulate -1e9
                            # into the diagonal 128-block (ident.T@negL)
                            nc.tensor.matmul(
                                sp[:, s, ds(128 * i, 128)],
                                ident_sb, negL_sb,
                                start=False, stop=True)
                    # one exp covers both heads of this key tile
                    nc.scalar.activation(
                        pt[:, :, lo:TCH], sp[:, :, lo:TCH],
                        Act.Exp, scale=1.0 / math.sqrt(D))
                    for s in range(2):
                        hs = 2 * p + s
                        nc.tensor.matmul(
                            pvps[s][:, lo:TCH],
                            v_sb[:, tk, hs, :],
                            pt[:, s, lo:TCH],
                            start=(tk == 0),
                            stop=(tk == n_tk - 1))
                    tick()
                for s in range(2):
                    # recip straight off PSUM, in parallel with the copy
                    # that releases the PSUM bank (pairs pipeline via psV)
                    r_sb = rsp.tile([1, TCH], bf16, tag="r")
                    nc.vector.reciprocal(
                        out=r_sb[:], in_=pvps[s][64:65, :])
                    ysb = rawp.tile([64, TCH], bf16, tag="ysb")
                    nc.vector.tensor_copy(out=ysb[:], in_=pvps[s][0:64, :])
                    rb = psC.tile([64, TCH], fp32, tag="c")
                    nc.tensor.matmul(
                        rb[:], ones64_sb, r_sb[:],
                        start=True, stop=True)
                    nc.vector.tensor_tensor(
                        out=yT[p][ds(64 * s, 64), ts(j, TCH)],
                        in0=ysb[:], in1=rb[:], op=Alu.mult)

            def phase_b_units(t, wide=False):
                """Output projection for T-tile t as 2 micro filler units;
                each half DMAs out as soon as its bias add lands. wide=True
                borrows the (idle, post-attention) psS pool for the
                accumulators so all final tiles pipeline in parallel."""
                osb = osbp.tile([128, C], fp32, tag="osb", name=f"osb{t}")

                def u_half(n):
                    if wide:
                        pot = psS.tile([128, 2, TCH], fp32, tag="sg")
                        po = pot[:, 0, :]
                    else:
                        po = psC.tile([128, TCH], fp32, tag="c")
                    for p in range(NPAIR):
                        nc.tensor.matmul(
                            po[:], yT[p][:, ts(t, 128)],
                            wp_sb[:, p, ts(n, TCH)],
                            start=(p == 0), stop=(p == NPAIR - 1))
                    nc.vector.tensor_tensor(
                        out=osb[:, ts(n, TCH)], in0=po[:],
                        in1=bpb_sb[:, ts(n, TCH)], op=Alu.add)
                    nc.sync.dma_start(
                        out_d[ts(t, 128), ts(n, TCH)], osb[:, ts(n, TCH)])

                return [lambda: u_half(0), lambda: u_half(1)]

            def phase_b(t):
                for u in phase_b_units(t):
                    u()

            # ---- emission ----
            # DMA order: x chunk 0 and wv first (first compute is the v
            # projection), then the other weights/constants as needed.
            pu0 = proj_units(0)
            pu0[0]()                        # xt0 DMA
            # wv in two halves so the first v matmuls start sooner
            wvr = wv_d.rearrange("(k q) f -> q k f", q=128)
            nc.sync.dma_start(wv_sb[:, 0:CK // 2, :], wvr[:, 0:CK // 2, :])
            nc.sync.dma_start(wv_sb[:, CK // 2:, :], wvr[:, CK // 2:, :])
            nc.sync.dma_start(
                wq_sb[:], wq_d.rearrange("(k q) f -> q k f", q=128))
            nc.sync.dma_start(cpak_sb[:], cpak_d[:, :])
            nc.sync.dma_start(hpak_sb[:], hpak_d[:, :])
            nc.sync.dma_start(
                wk_sb[:], wk_d.rearrange("(k q) f -> q k f", q=128))
            nc.vector.tensor_copy(
                out=v_sb[:, :, :, 64:65],
                in_=ones8_sb[:, None, :, None].to_broadcast(
                    (128, NTK, HPC, 1)))
            for u in pu0[1:]:
                u()
            nc.sync.dma_start(
                wp_sb[:], wp_d.rearrange("(p q) n -> q p n", q=128))
            # Filler schedule: projection chunks front-loaded as coarse
            # fillers between attention pairs (they must finish before
            # their own attention column); output-projection rows become
            # micro fillers ticked INSIDE the late, ACT-bound attention
            # columns so PE fill never starves ACT of score tiles.
            pu1, pu2, pu3 = (proj_units(1), proj_units(2), proj_units(3))
            coarse = {
                0: [],
                1: [],
                2: [],
                3: [],
            }
            micro = {
                0: pu1 + pu2,
                1: pu3[:9],
                2: pu3[9:] + [u for t in range(4) for u in phase_b_units(t)],
                3: [u for t in range(4, 12) for u in phase_b_units(t)],
            }
            for j in range(NTCH):
                fill = coarse[j]
                mic = micro[j]
                ticks_total = NPAIR * 2 * (2 * j + 2)
                stride = max(1, ticks_total // max(1, len(mic)))
                cnt = [0]

                def tick():
                    cnt[0] += 1
                    if mic and cnt[0] % stride == 0:
                        mic.pop(0)()

                k = len(fill)
                for p in range(NPAIR):
                    attention_col(p, j, tick)
                    take = (k * (p + 1)) // NPAIR - (k * p) // NPAIR
                    for _ in range(take):
                        fill.pop(0)()
                for u in mic:
                    u()
            for t in range(12, NTK):
                for u in phase_b_units(t, wide=(t % 2 == 1)):
                    u()
    _split_drain_waits(nc, mybir)
    return nc


def _split_drain_waits(nc, mybir, max_w=1):
    """This walrus build allows at most one embedded sync wait per
    instruction (CTRL_NO for drains, S3_LW for matmuls, ...). Hoist all but
    the last wait of every instruction into standalone EventSemaphore
    instructions on the same engine, inserted immediately before it."""
    import bass_rust

    for f in nc.m.functions:
        for blk in f.blocks:
            insts = list(blk.instructions)
            out = []
            changed = False
            for ins in insts:
                si = ins.sync_info
                if si is not None and si.on_wait and len(si.on_wait) > max_w:
                    changed = True
                    waits = list(si.on_wait)
                    extra, keep = waits[:-max_w], waits[-max_w:]
                    for wi, w in enumerate(extra):
                        ev = mybir.InstEventSemaphore(
                            name=f"{ins.name}_w{wi}",
                            engine=ins.engine,
                            ins=[], outs=[],
                            debug=ins.debug,
                            sync_info=bass_rust.SyncInfo(
                                on_wait=[w], on_update=[]),
                        )
                        nc.register_instruction(ev, overwrite=True)
                        out.append(ev)
                    si.on_wait = keep
                    ins.sync_info = si
                out.append(ins)
            if changed:
                blk.instructions = out


def host_inputs(x, Wq, bq, Wk, bk, Wv, bv, Wp, bp):
    """Build the 8 per-core input maps."""
    import ml_dtypes

    bft = ml_dtypes.bfloat16
    half = D // 2
    perm = np.concatenate([np.arange(0, D, 2), np.arange(1, D, 2)])  # even|odd
    pos = np.arange(T, dtype=np.float32)[:, None]
    freqs = np.exp(np.arange(half, dtype=np.float32)
                   * np.float32(-math.log(10000.0) / (half - 1)))[None, :]
    args = pos * freqs                      # [T, 32]
    cos = np.cos(args).astype(np.float32).T   # [32, T]
    sin = np.sin(args).astype(np.float32).T
    ropeA = np.concatenate([cos, cos, cos, cos], 0).astype(bft)   # [128, T]
    ropeB = np.concatenate([-sin, sin, -sin, sin], 0).astype(bft)
    pswap = np.zeros((128, 128), np.float32)
    for blk in range(4):
        b0 = 32 * blk
        src = 32 * (blk ^ 1)
        for i in range(32):
            pswap[b0 + i, src + i] = 1.0
    r_idx = np.arange(128)[:, None]
    c_idx = np.arange(128)[None, :]
    negL = np.where(r_idx > c_idx, -1.0e9, 0.0).astype(bft)
    ident = np.eye(128).astype(bft)

    in_maps = []
    for core in range(NCORES):
        b = core // 2
        h0 = (core % 2) * HPC
        cols = []
        for p in range(NPAIR):
            for hh in range(2):
                h = h0 + 2 * p + hh
                cols.append(h * D + perm)
        cols = np.concatenate(cols)           # deinterleaved q/k columns
        vcols = np.arange(h0 * D, (h0 + HPC) * D)
        bq_r = np.ascontiguousarray(
            bq[cols].reshape(NPAIR, 128).T)   # [128, 4]
        bk_r = np.ascontiguousarray(bk[cols].reshape(NPAIR, 128).T)
        bp_core = bp if core % 2 == 0 else np.zeros_like(bp)
        # fp32 pack: bq_r[4] | bk_r[4] | bv_b[512] | bp_b[1024]
        cpak = np.concatenate([
            bq_r, bk_r,
            np.broadcast_to(bv[vcols], (128, HPC * D)),
            np.broadcast_to(bp_core, (128, C)),
        ], axis=1).astype(np.float32)
        # bf16 pack: pswap[128] | ones8[8] | ones64row[64] | tri[128]
        #            | ropeA[2048] | ropeB[2048]  (ones64 row 0 only used)
        hpak = np.concatenate([
            pswap.astype(bft),
            np.ones((128, HPC + D), bft),
            negL, ident, ropeA, ropeB,
        ], axis=1).astype(bft)
        in_maps.append({
            "xT": np.ascontiguousarray(x[b].T).astype(bft),
            "wq": np.ascontiguousarray(Wq[:, cols]).astype(bft),
            "wk": np.ascontiguousarray(Wk[:, cols]).astype(bft),
            "wv": np.ascontiguousarray(Wv[:, vcols]).astype(bft),
            "wp": np.ascontiguousarray(Wp[vcols, :]).astype(bft),
            "cpak": cpak,
            "hpak": hpak,
        })
    return in_maps


_CACHE = {}
_PROFILE = False


def kernel(**inputs) -> np.ndarray:
    x = np.asarray(inputs["x"], np.float32)
    in_maps = host_inputs(
        x, *(np.asarray(inputs[k], np.float32) for k in
             ("Wq", "bq", "Wk", "bk", "Wv", "bv", "Wp", "bp")))
    from concourse.bass_utils import run_bass_kernel_spmd
    if "nc" not in _CACHE:
        _CACHE["nc"] = build_nc()
    bkr = run_bass_kernel_spmd(
        _CACHE["nc"], in_maps, core_ids=list(range(NCORES)),
        trace=_PROFILE)
    _CACHE["last"] = bkr
    res = bkr.results
    out = np.empty((B, T, C), np.float32)
    for b in range(B):
        out[b] = res[2 * b]["out"] + res[2 * b + 1]["out"]
    return out
